# revision 1
# baseline (speedup 1.0000x reference)
"""ChildSum Tree-LSTM on 8 Trainium2 NeuronCores (Bass/Tile).

Strategy: the tree (child_idx/child_mask) is input data, so the kernel builds an
input-specialized static schedule on the host: nodes are grouped into
level-synchronous steps (1261 steps for the reference tree), all gathers and
scatters are turned into contiguous-run copies via consumer-sorted layouts, and
the whole program is emitted fully unrolled.

Layout is "transposed" throughout: hidden dims live on SBUF partitions (4 chunks
of 128), nodes on the free axis — so the tiny per-step batches (<=32 nodes) run
elementwise ops on full 128-lane tiles. Recurrent matmuls are bf16
weight-stationary (lhsT = 128x128 weight tile, moving = node columns).

State movement:
  - leaf states   -> DRAM "leaf store" written in consumer-sorted order (bulk)
  - slack>=2 edges-> DRAM "long store" (consumer-sorted; run-writes at producer,
                     run-reads at consumer)
  - slack==1 edges-> direct SBUF copies from the previous step's output tiles
All runs/offsets are static; masked child slots stay zero (zero h and c
contribute nothing to child-sum or fc, matching the reference's masking).

All 8 cores run the identical program redundantly; core 0's output is returned.
"""
import sys
import numpy as np

sys.path.insert(0, "/opt/trn_rl_repo")

import ml_dtypes  # noqa: E402

Bcap = 32
H = 512
NCH = 4  # hidden chunks of 128


# --------------------------------------------------------------------------
# schedule construction (host, input-specialized)
# --------------------------------------------------------------------------
def build_schedule(child_idx, child_mask):
    N, K = child_idx.shape
    cnt = child_mask.sum(axis=1).astype(np.int64)
    kids = [[int(child_idx[p, s]) for s in range(int(cnt[p]))] for p in range(N)]

    level = np.zeros(N, dtype=np.int64)
    for i in range(N):
        if cnt[i]:
            level[i] = 1 + max(level[j] for j in kids[i])
    depth = int(level.max())
    is_leaf = cnt == 0

    steps = []
    for L in range(1, depth + 1):
        nodes = [int(n) for n in np.where(level == L)[0]]
        for i in range(0, len(nodes), Bcap):
            steps.append(nodes[i:i + Bcap])
    T = len(steps)
    step_of = -np.ones(N, np.int64)
    for t, nodes in enumerate(steps):
        for n in nodes:
            step_of[n] = t

    slot_children = {}
    for p in range(N):
        if cnt[p]:
            slot_children[p] = sorted(kids[p], key=lambda c: -step_of[c])

    pos_in_step = {}
    for t, nodes in enumerate(steps):
        for i, p in enumerate(nodes):
            pos_in_step[p] = i
    BIGKEY = (10**9, 0, 0)
    for _ in range(4):
        cons_key = {}
        for t, nodes in enumerate(steps):
            for p in nodes:
                for s, c in enumerate(slot_children[p]):
                    cons_key[c] = (t, s, pos_in_step[p])
        for t, nodes in enumerate(steps):
            nodes.sort(key=lambda n: cons_key.get(n, BIGKEY))
            for i, p in enumerate(nodes):
                pos_in_step[p] = i
    cons_key = {}
    for t, nodes in enumerate(steps):
        for p in nodes:
            for s, c in enumerate(slot_children[p]):
                cons_key[c] = (t, s, pos_in_step[p])

    sizes = np.array([len(s) for s in steps], dtype=np.int64)
    cum = np.zeros(T + 1, dtype=np.int64)
    cum[1:] = np.cumsum(sizes)
    NI = int(cum[T])

    leaf_edges, s1_edges, long_edges = [], [], []
    for c, (t, s, pp) in cons_key.items():
        if is_leaf[c]:
            leaf_edges.append((t, s, pp, c))
        elif step_of[c] == t - 1:
            s1_edges.append((t, s, pp, c))
        else:
            long_edges.append((t, s, pp, c))
    leaf_edges.sort(); s1_edges.sort(); long_edges.sort()

    leaf_order = [c for (_, _, _, c) in leaf_edges]
    att = set(leaf_order)
    leaf_order += [int(c) for c in np.where(is_leaf)[0] if int(c) not in att]
    leaf_lpos = {c: i for i, c in enumerate(leaf_order)}
    NL = len(leaf_order)

    long_order = [c for (_, _, _, c) in long_edges]
    long_lpos = {c: i for i, c in enumerate(long_order)}
    NLong = max(1, len(long_order))

    def runs(edges, srcf):
        per = {}
        cur = None
        out = []
        for (t, s, pp, c) in edges:
            sp = srcf(c)
            if cur and cur[0] == t and cur[3] == s and sp == cur[1] + cur[4] and pp == cur[2] + cur[4]:
                cur[4] += 1
            else:
                if cur:
                    out.append(cur)
                cur = [t, sp, pp, s, 1]
        if cur:
            out.append(cur)
        for r in out:
            per.setdefault(r[0], []).append((r[1], r[2], r[3], r[4]))
        return per

    s1_runs = runs(s1_edges, lambda c: pos_in_step[c])
    leaf_read_runs = runs(leaf_edges, lambda c: leaf_lpos[c])
    long_read_runs = runs(long_edges, lambda c: long_lpos[c])

    lw = sorted((int(step_of[c]), pos_in_step[c], long_lpos[c]) for c in long_order)
    long_write_runs = {}
    cur = None
    out = []
    for (t, lp, ld) in lw:
        if cur and cur[0] == t and lp == cur[1] + cur[3] and ld == cur[2] + cur[3]:
            cur[3] += 1
        else:
            if cur:
                out.append(cur)
            cur = [t, lp, ld, 1]
    if cur:
        out.append(cur)
    for r in out:
        long_write_runs.setdefault(r[0], []).append(tuple(r[1:]))

    PAD = 512
    NLp = (NL + PAD - 1) // PAD * PAD
    NIp = (NI + PAD - 1) // PAD * PAD
    int_order = [p for t in range(T) for p in steps[t]]
    p1_order = np.zeros(NLp + NIp, dtype=np.int64)
    p1_order[:NL] = leaf_order
    p1_order[NLp:NLp + NI] = int_order

    return dict(
        N=N, steps=steps, T=T, sizes=sizes, cum=cum,
        s1_runs=s1_runs, leaf_read_runs=leaf_read_runs,
        long_read_runs=long_read_runs, long_write_runs=long_write_runs,
        leaf_order=leaf_order, NL=NL, NLp=NLp, NI=NI, NIp=NIp,
        NLong=NLong, p1_order=p1_order, int_order=int_order,
    )


# --------------------------------------------------------------------------
# device program
# --------------------------------------------------------------------------
def build_program(S):
    import concourse.bacc as bacc
    import concourse.mybir as mybir
    from concourse import tile

    dt = mybir.dt
    Act = mybir.ActivationFunctionType
    Alu = mybir.AluOpType

    NP = S["NLp"] + S["NIp"]
    NB = NP // 512
    NLB = S["NLp"] // 512

    nc = bacc.Bacc("TRN2", target_bir_lowering=False, debug=False, num_devices=8)

    xT = nc.dram_tensor("xT", [128, NCH, NP], dt.float32r, kind="ExternalInput")
    w1t = nc.dram_tensor("w1t", [128, 64, 128], dt.float32r, kind="ExternalInput")
    w2t = nc.dram_tensor("w2t", [128, 64, 128], dt.float32, kind="ExternalInput")
    biasc = nc.dram_tensor("biasc", [128, 16], dt.float32, kind="ExternalInput")

    lf_h = nc.dram_tensor("lf_h", [128, NCH, S["NLp"]], dt.float32, kind="ExternalOutput")
    lf_c = nc.dram_tensor("lf_c", [128, NCH, S["NLp"]], dt.float32, kind="ExternalOutput")
    out_h = nc.dram_tensor("out_h", [128, NCH, S["NIp"]], dt.float32, kind="ExternalOutput")
    out_c = nc.dram_tensor("out_c", [128, NCH, S["NIp"]], dt.float32, kind="ExternalOutput")

    p1out = nc.dram_tensor("p1out", [128, 16, S["NIp"]], dt.float32)
    lg_h = nc.dram_tensor("lg_h", [128, NCH, S["NLong"]], dt.float32)
    lg_c = nc.dram_tensor("lg_c", [128, NCH, S["NLong"]], dt.float32)

    with tile.TileContext(nc, trace_sim=False) as tc:
        with tc.tile_pool(name="wp", bufs=1) as wp:
            w1sb = wp.tile([128, 64, 128], dt.float32r, tag="w1")
            nc.sync.dma_start(w1sb[:], w1t.ap())
            w2sb = wp.tile([128, 64, 128], dt.float32, tag="w2")
            nc.sync.dma_start(w2sb[:], w2t.ap())
            bsb = wp.tile([128, 16], dt.float32, tag="bias")
            nc.sync.dma_start(bsb[:], biasc.ap())

            # ---------------- phase 1: i2h projection (+ leaf gates inline)
            with (
                tc.tile_pool(name="xp", bufs=3) as xp,
                tc.tile_pool(name="stp", bufs=1) as stp,
                tc.tile_pool(name="p1ps", bufs=4, space="PSUM") as p1ps,
                tc.tile_pool(name="lgp", bufs=1) as lgp,
            ):
                for nb in range(NB):
                    xt = xp.tile([128, NCH, 512], dt.float32r, tag="xt")
                    nc.sync.dma_start(xt[:], xT.ap()[:, :, nb * 512:(nb + 1) * 512])
                    stag = stp.tile([128, 16, 512], dt.float32, tag="stag")
                    for m in range(16):
                        ps = p1ps.tile([128, 512], dt.float32, tag="p1")
                        for k in range(NCH):
                            nc.tensor.matmul(ps[:], w1sb[:, m * 4 + k, :], xt[:, k, :],
                                             start=(k == 0), stop=(k == NCH - 1))
                        # evacuate psum + add folded bias (per-partition scalar)
                        nc.vector.tensor_scalar_add(stag[:, m, :], ps[:],
                                                    bsb[:, m:m + 1])
                    if nb < NLB:
                        # leaf gates: c = sig(i)*tanh(u); h = sig(o)*tanh(c)
                        sio = lgp.tile([128, 8, 512], dt.float32, tag="sio")
                        nc.scalar.activation(sio[:], stag[:, 0:8, :], Act.Sigmoid)
                        tu = lgp.tile([128, NCH, 512], dt.float32, tag="tu")
                        nc.scalar.activation(tu[:], stag[:, 8:12, :], Act.Tanh)
                        cst = lgp.tile([128, NCH, 512], dt.float32, tag="cst")
                        nc.vector.tensor_tensor(cst[:], sio[:, 0:4, :], tu[:], op=Alu.mult)
                        nc.sync.dma_start(lf_c.ap()[:, :, nb * 512:(nb + 1) * 512], cst[:])
                        tcl = lgp.tile([128, NCH, 512], dt.float32, tag="tcl")
                        nc.scalar.activation(tcl[:], cst[:], Act.Tanh)
                        hst = lgp.tile([128, NCH, 512], dt.float32, tag="hst")
                        nc.vector.tensor_tensor(hst[:], sio[:, 4:8, :], tcl[:], op=Alu.mult)
                        nc.sync.dma_start(lf_h.ap()[:, :, nb * 512:(nb + 1) * 512], hst[:])
                    else:
                        ib = nb - NLB
                        nc.sync.dma_start(p1out.ap()[:, :, ib * 512:(ib + 1) * 512], stag[:])

            # ---------------- recurrence
            with (
                tc.tile_pool(name="prep", bufs=6) as prep,
                tc.tile_pool(name="cbtp", bufs=6) as cbtp,
                tc.tile_pool(name="rps", bufs=2, space="PSUM") as rps,
                tc.tile_pool(name="wk", bufs=3) as wk,
                tc.tile_pool(name="hc", bufs=3) as hc,
            ):
                prev_h = prev_c = None
                cum = S["cum"]
                for t in range(S["T"]):
                    B = int(S["sizes"][t])
                    off = int(cum[t])
                    pre = prep.tile([128, 16, B], dt.float32, tag="pre")
                    nc.sync.dma_start(pre[:], p1out.ap()[:, :, off:off + B])

                    cbth = cbtp.tile([128, NCH, 4, B], dt.float32, tag="cbth")
                    cbtc = cbtp.tile([128, NCH, 4, B], dt.float32, tag="cbtc")
                    nc.gpsimd.memset(cbth[:], 0)
                    nc.gpsimd.memset(cbtc[:], 0)
                    for (src, pp, s, ln) in S["leaf_read_runs"].get(t, []):
                        nc.sync.dma_start(cbth[:, :, s, pp:pp + ln], lf_h.ap()[:, :, src:src + ln])
                        nc.sync.dma_start(cbtc[:, :, s, pp:pp + ln], lf_c.ap()[:, :, src:src + ln])
                    for (src, pp, s, ln) in S["long_read_runs"].get(t, []):
                        nc.sync.dma_start(cbth[:, :, s, pp:pp + ln], lg_h.ap()[:, :, src:src + ln])
                        nc.sync.dma_start(cbtc[:, :, s, pp:pp + ln], lg_c.ap()[:, :, src:src + ln])
                    for (src, pp, s, ln) in S["s1_runs"].get(t, []):
                        nc.vector.tensor_copy(cbth[:, :, s, pp:pp + ln], prev_h[:, :, src:src + ln])
                        nc.gpsimd.tensor_copy(cbtc[:, :, s, pp:pp + ln], prev_c[:, :, src:src + ln])

                    # hs = sum over slots (bf16)
                    hs1 = wk.tile([128, NCH, B], dt.float32, tag="hs1")
                    hs2 = wk.tile([128, NCH, B], dt.float32, tag="hs2")
                    hst = wk.tile([128, NCH, B], dt.float32, tag="hst")
                    nc.gpsimd.tensor_tensor(hs1[:], cbth[:, :, 0, :], cbth[:, :, 1, :], op=Alu.add)
                    nc.gpsimd.tensor_tensor(hs2[:], cbth[:, :, 2, :], cbth[:, :, 3, :], op=Alu.add)
                    nc.gpsimd.tensor_tensor(hst[:], hs1[:], hs2[:], op=Alu.add)

                    ps_iuo = rps.tile([128, 12, B], dt.float32, tag="iuo")
                    ps_f = rps.tile([128, NCH, 4, B], dt.float32, tag="f")
                    for m in range(12):
                        for k in range(NCH):
                            nc.tensor.matmul(ps_iuo[:, m, :], w2sb[:, m * 4 + k, :],
                                             hst[:, k, :], start=(k == 0), stop=(k == 3))
                    for m in range(NCH):
                        for k in range(NCH):
                            nc.tensor.matmul(ps_f[:, m, :, :], w2sb[:, 48 + m * 4 + k, :],
                                             cbth[:, k, :, :], start=(k == 0), stop=(k == 3))

                    # f gate
                    fp4 = wk.tile([128, NCH, 4, B], dt.float32, tag="fp4")
                    for s in range(4):
                        nc.scalar.activation(fp4[:, :, s, :], pre[:, 12:16, :], Act.Copy)
                    fsum = wk.tile([128, NCH, 4, B], dt.float32, tag="fsum")
                    nc.vector.tensor_tensor(fsum[:], ps_f[:], fp4[:], op=Alu.add)
                    ft = wk.tile([128, NCH, 4, B], dt.float32, tag="ft")
                    nc.scalar.activation(ft[:], fsum[:], Act.Sigmoid)
                    pt = wk.tile([128, NCH, 4, B], dt.float32, tag="pt")
                    nc.vector.tensor_tensor(pt[:], ft[:], cbtc[:], op=Alu.mult)
                    fc1 = wk.tile([128, NCH, B], dt.float32, tag="fc1")
                    fc2 = wk.tile([128, NCH, B], dt.float32, tag="fc2")
                    fct = wk.tile([128, NCH, B], dt.float32, tag="fct")
                    nc.gpsimd.tensor_tensor(fc1[:], pt[:, :, 0, :], pt[:, :, 1, :], op=Alu.add)
                    nc.gpsimd.tensor_tensor(fc2[:], pt[:, :, 2, :], pt[:, :, 3, :], op=Alu.add)
                    nc.gpsimd.tensor_tensor(fct[:], fc1[:], fc2[:], op=Alu.add)

                    # iuo gates
                    gt = wk.tile([128, 12, B], dt.float32, tag="gt")
                    nc.vector.tensor_tensor(gt[:], ps_iuo[:], pre[:, 0:12, :], op=Alu.add)
                    sio = wk.tile([128, 8, B], dt.float32, tag="sio")
                    nc.scalar.activation(sio[:], gt[:, 0:8, :], Act.Sigmoid)
                    tu = wk.tile([128, NCH, B], dt.float32, tag="tu")
                    nc.scalar.activation(tu[:], gt[:, 8:12, :], Act.Tanh)
                    gg = wk.tile([128, NCH, B], dt.float32, tag="gg")
                    nc.vector.tensor_tensor(gg[:], sio[:, 0:4, :], tu[:], op=Alu.mult)
                    ct = hc.tile([128, NCH, B], dt.float32, tag="ct")
                    nc.vector.tensor_tensor(ct[:], gg[:], fct[:], op=Alu.add)
                    tct = wk.tile([128, NCH, B], dt.float32, tag="tct")
                    nc.scalar.activation(tct[:], ct[:], Act.Tanh)
                    ht = hc.tile([128, NCH, B], dt.float32, tag="ht")
                    nc.vector.tensor_tensor(ht[:], sio[:, 4:8, :], tct[:], op=Alu.mult)

                    nc.sync.dma_start(out_h.ap()[:, :, off:off + B], ht[:])
                    nc.sync.dma_start(out_c.ap()[:, :, off:off + B], ct[:])
                    for (lp, ld, ln) in S["long_write_runs"].get(t, []):
                        nc.sync.dma_start(lg_h.ap()[:, :, ld:ld + ln].opt(), ht[:, :, lp:lp + ln].opt())
                        nc.sync.dma_start(lg_c.ap()[:, :, ld:ld + ln].opt(), ct[:, :, lp:lp + ln].opt())
                    prev_h, prev_c = ht, ct

    nc.compile()
    return nc


# --------------------------------------------------------------------------
# host data prep + entry point
# --------------------------------------------------------------------------
def prep_inputs(S, inputs, i2h_weight, i2h_bias, hs2h_weight, hs2h_bias,
                hc2h_weight, hc2h_bias):
    NP = S["NLp"] + S["NIp"]
    # gate-reordered [i, o, u, f] big weight and folded bias
    Wb = np.concatenate([i2h_weight[0:H], i2h_weight[3 * H:4 * H],
                         i2h_weight[2 * H:3 * H], i2h_weight[1 * H:2 * H]], 0)
    bb = np.concatenate([i2h_bias[0:H] + hs2h_bias[0:H],
                         i2h_bias[3 * H:4 * H] + hs2h_bias[2 * H:3 * H],
                         i2h_bias[2 * H:3 * H] + hs2h_bias[1 * H:2 * H],
                         i2h_bias[1 * H:2 * H] + hc2h_bias], 0)
    x_perm = inputs[S["p1_order"]]                      # [NP, 512]
    xTa = np.ascontiguousarray(
        x_perm.T.reshape(NCH, 128, NP).transpose(1, 0, 2)).astype(np.float32)

    w1 = np.zeros((128, 64, 128), dtype=np.float32)
    for m in range(16):
        for k in range(NCH):
            blk = Wb[m * 128:(m + 1) * 128, k * 128:(k + 1) * 128]
            w1[:, m * 4 + k, :] = blk.T.astype(np.float32)

    Whs_r = np.concatenate([hs2h_weight[0:H], hs2h_weight[2 * H:3 * H],
                            hs2h_weight[1 * H:2 * H]], 0)  # [i,o,u] rows
    w2 = np.zeros((128, 64, 128), dtype=np.float32)
    for m in range(12):
        for k in range(NCH):
            blk = Whs_r[m * 128:(m + 1) * 128, k * 128:(k + 1) * 128]
            w2[:, m * 4 + k, :] = blk.T.astype(np.float32)
    for m in range(NCH):
        for k in range(NCH):
            blk = hc2h_weight[k * 128:(k + 1) * 128, m * 128:(m + 1) * 128]
            w2[:, 48 + m * 4 + k, :] = blk.astype(np.float32)

    bcol = bb.reshape(16, 128).T.astype(np.float32).copy()  # [128, 16]
    return {"xT": xTa, "w1t": w1, "w2t": w2, "biasc": bcol}


def assemble(S, res):
    N = S["N"]
    Hh = np.zeros((N, H), dtype=np.float32)
    Cc = np.zeros((N, H), dtype=np.float32)

    def untile(a):  # [128, 4, M] -> [M, 512]
        return np.ascontiguousarray(a.astype(np.float32).transpose(2, 1, 0).reshape(a.shape[2], H))

    lfh = untile(res["lf_h"]); lfc = untile(res["lf_c"])
    oh = untile(res["out_h"]); oc = untile(res["out_c"])
    lo = np.array(S["leaf_order"], dtype=np.int64)
    Hh[lo] = lfh[:S["NL"]]; Cc[lo] = lfc[:S["NL"]]
    io = np.array(S["int_order"], dtype=np.int64)
    Hh[io] = oh[:S["NI"]]; Cc[io] = oc[:S["NI"]]
    return np.stack([Hh, Cc]).astype(np.float32)


_CACHE = {}


def kernel(inputs, i2h_weight, i2h_bias, hs2h_weight, hs2h_bias,
           hc2h_weight, hc2h_bias, child_idx, child_mask):
    from concourse.bass_utils import run_bass_kernel_spmd

    inputs = np.asarray(inputs, dtype=np.float32)
    i2h_weight = np.asarray(i2h_weight, dtype=np.float32)
    i2h_bias = np.asarray(i2h_bias, dtype=np.float32)
    hs2h_weight = np.asarray(hs2h_weight, dtype=np.float32)
    hs2h_bias = np.asarray(hs2h_bias, dtype=np.float32)
    hc2h_weight = np.asarray(hc2h_weight, dtype=np.float32)
    hc2h_bias = np.asarray(hc2h_bias, dtype=np.float32)
    child_idx = np.asarray(child_idx, dtype=np.int32)
    child_mask = np.asarray(child_mask, dtype=np.int32)

    key = (child_idx.tobytes(), child_mask.tobytes())
    if key not in _CACHE:
        S = build_schedule(child_idx, child_mask)
        ncp = build_program(S)
        _CACHE[key] = (S, ncp)
    S, ncp = _CACHE[key]

    inm = prep_inputs(S, inputs, i2h_weight, i2h_bias, hs2h_weight, hs2h_bias,
                      hc2h_weight, hc2h_bias)
    res = run_bass_kernel_spmd(ncp, [inm] * 8, list(range(8)))
    return assemble(S, res.results[0])

